# revision 23
# baseline (speedup 1.0000x reference)
"""MultiHeadedAttention Trainium2 Bass kernel (v2: fp8-DR PV + split exp).

Full inputs in, full output out. Sharding: 8 cores = 4 batches x 2 head-pairs.
Per core (1 batch, 2 heads):
  Q/K projections bf16 -> [128, 2048] bf16 SBUF (ACT copy w/ bias, fp32 PSUM)
  V^T projection -> vt [128 m, 16 mb, 2 h, 128] fp8e4m3 where cols 0:64 = v^T
  and cols 64:128 = ones (softmax sums come out of the PV matmul pre-broadcast
  to 64 partitions -- zero extra cycles, matmul cost is N-driven).
  Attention per (n-window 512, head): m-loop over 8 mb-pairs:
    scores bf16 [K=64, M=128, N=512] x2 -> PSUM [128, 2, 512]
    exp: split between ACT (native Exp, fp8 out, scale=1/8) and DVE
    (exp2 bit-trick: i8 = round(s*log2e*8/8 + C2), bits reinterpreted as
    e4m3 = 2^((i8-56)/8) ~ exp(s/8), one tensor_scalar per tile)
    PV-accum: fp8 DoubleRow matmul, K=2x128, accumulated over 8 pairs.
  normalize: single DVE divide px[0:64]/px[64:128] -> x bf16
  out projection bf16, heads accumulated in PSUM; bf16 partial out.
Host sums the two per-batch partials and adds the output bias in fp32.
"""

import sys

if "/opt/trn_rl_repo" not in sys.path:
    sys.path.insert(0, "/opt/trn_rl_repo")

import numpy as np
import ml_dtypes

BF = ml_dtypes.bfloat16

B, D, N, H = 4, 256, 2048, 4
DIM = D // H  # 64
NW = 4  # 512-wide n windows
MB = 16  # 128-wide m blocks

# exp2 bit-trick constant: i8 = round(s * log2(e) + C2); tuned on the
# reference distribution (see transcript prototype).
C1 = 1.4426950408889634
C2 = 55.75

# exp engine pattern per (nw, h) block of 8 mb-pairs: True = ACT, False = DVE.
# j=7 kept on DVE so the div (DVE) queued right after never head-blocks.
PAT_EVEN = [True, False, True, False, True, True, True, False]   # 5A/3D
PAT_ODD = [True, False, True, False, True, False, True, False]   # 4A/4D

_CACHE = {}


def _emit(ctx, tc, io):
    import concourse.bass as bass
    import concourse.mybir as mybir

    nc = tc.nc
    f32 = mybir.dt.float32
    bf16 = mybir.dt.bfloat16
    fp8 = mybir.dt.float8e4
    i8 = mybir.dt.int8
    EXP = mybir.ActivationFunctionType.Exp
    COPY = mybir.ActivationFunctionType.Copy
    IDENT = mybir.ActivationFunctionType.Identity
    DR = mybir.MatmulPerfMode.DoubleRow
    DIVIDE = mybir.AluOpType.divide

    const = ctx.enter_context(tc.tile_pool(name="const", bufs=1))
    xin = ctx.enter_context(tc.tile_pool(name="xin", bufs=4))
    big = ctx.enter_context(tc.tile_pool(name="big", bufs=1))
    ptp = ctx.enter_context(tc.tile_pool(name="probs", bufs=4))
    otp = ctx.enter_context(tc.tile_pool(name="outp", bufs=2))
    psS = ctx.enter_context(tc.tile_pool(name="psS", bufs=3, space="PSUM"))
    psX = ctx.enter_context(tc.tile_pool(name="psX", bufs=2, space="PSUM"))

    # ---- constants / weights ----
    wqt_sb = const.tile([128, 2, 128], bf16, tag="wqt")
    nc.sync.dma_start(wqt_sb, io["wqt"].rearrange("(c p) o -> p c o", p=128))
    wkt_sb = const.tile([128, 2, 128], bf16, tag="wkt")
    nc.sync.dma_start(wkt_sb, io["wkt"].rearrange("(c p) o -> p c o", p=128))
    wvt_sb = const.tile([128, 2, 128], bf16, tag="wvt")
    nc.sync.dma_start(wvt_sb, io["wvt"].rearrange("(c p) o -> p c o", p=128))
    wmt0_sb = const.tile([64, 256], bf16, tag="wmt0")
    nc.sync.dma_start(wmt0_sb, io["wmt0"])
    wmt1_sb = const.tile([64, 256], bf16, tag="wmt1")
    nc.sync.dma_start(wmt1_sb, io["wmt1"])
    bq_sb = const.tile([128, 1], f32, tag="bq")
    nc.sync.dma_start(bq_sb, io["bq"])
    bk_sb = const.tile([128, 1], f32, tag="bk")
    nc.sync.dma_start(bk_sb, io["bk"])
    bv_sb = const.tile([1, 128], bf16, tag="bv")
    nc.sync.dma_start(bv_sb, io["bv"])
    onesb = const.tile([1, 128], bf16, tag="onesb")
    nc.gpsimd.memset(onesb, 1.0)

    # v^T in fp8: [m-in-block, mb, head, col]; cols 64:128 are the ones block
    # that makes the PV matmul emit softmax sums broadcast to partitions 64:128.
    vt = big.tile([128, MB, 2, 128], fp8, tag="vt")
    nc.gpsimd.memset(vt[:, :, :, 64:128], 1.0)

    # PE warmup: release the HAM clock gate during the input-DMA ramp.
    wu_a = const.tile([128, 128], bf16, tag="wu_a")
    nc.gpsimd.memset(wu_a, 0.0)
    wu_b = const.tile([128, 512], bf16, tag="wu_b")
    nc.gpsimd.memset(wu_b, 0.0)
    wu_ps = psS.tile([128, 2, 512], f32, tag="ps", name="wu_ps")
    for _ in range(16):
        nc.tensor.matmul(wu_ps[:, 0, :], lhsT=wu_a, rhs=wu_b, start=True, stop=True)

    # ---- input loads ----
    xq_t, xk_t, xv_t = [], [], []
    eng = {"xq": nc.sync, "xk": nc.gpsimd, "xv": nc.scalar}
    for w in range(NW):
        for name, lst in (("xk", xk_t), ("xq", xq_t), ("xv", xv_t)):
            t = xin.tile([128, 2, 512], bf16, tag=name, name=f"{name}{w}")
            src = io[name].rearrange("(c p) n -> p c n", p=128)
            for hh in range(2):
                s = slice(w * 512 + hh * 256, w * 512 + (hh + 1) * 256)
                eng[name].dma_start(t[:, :, hh * 256 : (hh + 1) * 256], src[:, :, s])
            lst.append(t)

    # ---- projections ----
    q_sb = big.tile([128, 2048], bf16, tag="q")
    k_sb = big.tile([128, 2048], bf16, tag="k")

    def proj_pair(xt, wt, bias, dst, wlo):
        # two 512-n windows into one [128, 2, 512] psum tile, one ACT copy
        ps = psS.tile([128, 2, 512], f32, tag="ps", name=f"psproj{wlo}")
        for i, w in enumerate((wlo, wlo + 1)):
            nc.tensor.matmul(ps[:, i, :], lhsT=wt[:, 0, :], rhs=xt[w][:, 0, :],
                             start=True, stop=False)
            nc.tensor.matmul(ps[:, i, :], lhsT=wt[:, 1, :], rhs=xt[w][:, 1, :],
                             start=False, stop=True)
        nc.scalar.activation(
            dst[:, wlo * 512 : (wlo + 2) * 512].rearrange("p (i n) -> p i n", i=2),
            ps, IDENT, bias=bias)

    def vt_group(g):
        # 4 m-blocks (one xv window) -> one [128, 512] psum tile -> 2 fp8 copies
        grp = psX.tile([128, 512], f32, tag="px", name=f"psvt{g}")
        for i in range(4):
            pvt = grp[:, i * 128 : (i + 1) * 128]
            nc.tensor.matmul(pvt, lhsT=onesb, rhs=bv_sb, start=True, stop=False)
            nc.tensor.matmul(pvt, lhsT=xv_t[g][:, 0, i * 128 : (i + 1) * 128],
                             rhs=wvt_sb[:, 0, :], start=False, stop=False)
            nc.tensor.matmul(pvt, lhsT=xv_t[g][:, 1, i * 128 : (i + 1) * 128],
                             rhs=wvt_sb[:, 1, :], start=False, stop=True)
        gview = grp.rearrange("p (i c) -> p i c", i=4)
        for h in range(2):
            nc.scalar.activation(vt[:, 4 * g : 4 * g + 4, h, 0:64],
                                 gview[:, :, h * 64 : (h + 1) * 64], COPY)

    # ---- attention ----
    xh = [big.tile([64, 2048], bf16, tag=f"x{h}", name=f"x{h}") for h in range(2)]
    pend_div = [None]

    def flush_div():
        if pend_div[0] is not None:
            h, nw, px = pend_div[0]
            # sums sit pre-broadcast in px[64:128]; DVE has one PSUM read port
            # and no divide, so: ACT copies sums to SBUF, idle GPSIMD takes
            # the reciprocal (pow -1), DVE multiplies.
            s_sb = otp.tile([64, 512], f32, tag="s", name=f"s{h}_{nw}")
            nc.scalar.activation(s_sb, px[64:128, :], COPY)
            r_sb = otp.tile([64, 512], f32, tag="r", name=f"r{h}_{nw}")
            nc.vector.reciprocal_approx_fast(r_sb, s_sb)
            nc.vector.tensor_tensor(
                xh[h][:, nw * 512 : (nw + 1) * 512],
                px[0:64, :], r_sb, op=mybir.AluOpType.mult)
            pend_div[0] = None

    def make_block(nw, h, blk):
        return {"nw": nw, "h": h,
                "pat": PAT_EVEN if blk % 2 == 0 else PAT_ODD,
                "px": psX.tile([128, 512], f32, tag="px", name=f"px{blk}"),
                "pts": [], "blk": blk}

    # fillers: tiny junk matmuls into a provably-dead scores bank (its exp is
    # already complete when the filler issues, so no new semaphore waits).
    # They bridge the PE's exp-wait micro-gaps so the HAM activity window
    # never sees idle and the 2.4GHz clock gate stays open.
    dead_sc = [None]

    def filler(n=1):
        if dead_sc[0] is None:
            return
        for _ in range(n):
            nc.tensor.matmul(dead_sc[0][0:1, 0, 0:64], lhsT=wu_a[0:1, 0:1],
                             rhs=wu_b[0:1, 0:64], start=True, stop=True,
                             skip_group_check=True)

    def attn_pairs(st, j_lo, j_hi):
        nw, h, blk, px = st["nw"], st["h"], st["blk"], st["px"]
        kh = k_sb[h * 64 : (h + 1) * 64, :]
        qh = q_sb[h * 64 : (h + 1) * 64, nw * 512 : (nw + 1) * 512]
        for j in range(j_lo, j_hi):
            sc = psS.tile([128, 2, 512], f32, tag="ps", name=f"sc{blk}_{j}")
            st.setdefault("scs", []).append(sc)
            for kt in range(2):
                mb = 2 * j + kt
                nc.tensor.matmul(sc[:, kt, :],
                                 lhsT=kh[:, mb * 128 : (mb + 1) * 128], rhs=qh,
                                 start=True, stop=True)
            pt = ptp.tile([128, 2, 512], fp8, tag="pt", name=f"pt{blk}_{j}")
            if st["pat"][j]:
                nc.scalar.activation(pt, sc, EXP, scale=0.125)
            else:
                nc.vector.tensor_scalar(pt.bitcast(i8), sc, C1, C2,
                                        op0=mybir.AluOpType.mult,
                                        op1=mybir.AluOpType.add)
            st["pts"].append(pt)
            if j == 1:
                flush_div()
            filler(1)
            if j >= 2:
                jj = j - 2
                nc.tensor.matmul(px, lhsT=vt[:, 2 * jj : 2 * jj + 2, h, :],
                                 rhs=st["pts"][jj], start=(j == 2), stop=False,
                                 perf_mode=DR, skip_group_check=True)
                dead_sc[0] = st["scs"][jj]
                filler(1)

    def attn_tail(st):
        h, px = st["h"], st["px"]
        for j in (6, 7):
            nc.tensor.matmul(px, lhsT=vt[:, 2 * j : 2 * j + 2, h, :],
                             rhs=st["pts"][j], start=False, stop=(j == 7),
                             perf_mode=DR, skip_group_check=True)
            dead_sc[0] = st["scs"][j]
            filler(1)
        pend_div[0] = (st["h"], st["nw"], px)

    def attn_block(nw, h, blk):
        st = make_block(nw, h, blk)
        attn_pairs(st, 0, 8)
        attn_tail(st)

    def out_proj(half):
        for oc in range(2):
            po = psS.tile([128, 2, 512], f32, tag="ps", name=f"po{half}_{oc}")
            ocs = slice(oc * 128, (oc + 1) * 128)
            for s in range(2):
                ns = slice(half * 1024 + s * 512, half * 1024 + (s + 1) * 512)
                nc.tensor.matmul(po[:, s, :], lhsT=wmt0_sb[:, ocs],
                                 rhs=xh[0][:, ns], start=True, stop=False)
                nc.tensor.matmul(po[:, s, :], lhsT=wmt1_sb[:, ocs],
                                 rhs=xh[1][:, ns], start=False, stop=True)
            ot = otp.tile([128, 1024], bf16, tag="ot", name=f"ot{half}_{oc}")
            if oc == 0:
                nc.scalar.activation(ot, po.rearrange("p i n -> p (i n)"), COPY)
            else:
                nc.vector.tensor_copy(ot, po.rearrange("p i n -> p (i n)"))
            nc.sync.dma_start(
                io["out"][ocs, half * 1024 : (half + 1) * 1024], ot)

    # All projections and v^T groups first (they gate only on input DMA), then
    # a junk bridge covering the first q/k copy-wait so the HAM clock gate
    # stays open into the attention phase.
    proj_pair(xk_t, wkt_sb, bk_sb, k_sb, 0)
    proj_pair(xq_t, wqt_sb, bq_sb, q_sb, 0)
    vt_group(0)
    proj_pair(xk_t, wkt_sb, bk_sb, k_sb, 2)
    proj_pair(xq_t, wqt_sb, bq_sb, q_sb, 2)
    vt_group(1)
    vt_group(2)
    vt_group(3)
    br = psS.tile([128, 2, 512], f32, tag="ps", name="bridge")
    for _ in range(6):
        nc.tensor.matmul(br[:, 0, :], lhsT=wu_a, rhs=wu_b, start=True, stop=True)

    blk = 0
    for nw in range(NW):
        for h in range(2):
            attn_block(nw, h, blk)
            blk += 1
        if nw == 1:
            flush_div()
            out_proj(0)
    flush_div()
    out_proj(1)

    if "dbg_q" in io:
        nc.sync.dma_start(io["dbg_q"], q_sb)
        nc.sync.dma_start(io["dbg_k"], k_sb)
        nc.sync.dma_start(io["dbg_vt"], vt)
        nc.sync.dma_start(io["dbg_x0"], xh[0])
        nc.sync.dma_start(io["dbg_x1"], xh[1])


def _build_nc(debug_dumps=False):
    key = ("nc", debug_dumps)
    if key in _CACHE:
        return _CACHE[key]
    from contextlib import ExitStack

    import concourse.mybir as mybir
    import concourse.tile as tile
    from concourse import bacc

    f32 = mybir.dt.float32
    bf16 = mybir.dt.bfloat16
    fp8 = mybir.dt.float8e4
    nc = bacc.Bacc("TRN2", target_bir_lowering=False, debug=False, num_devices=8)
    io = {}
    for name, shape, dt_ in (
        ("xq", [256, 2048], bf16),
        ("xk", [256, 2048], bf16),
        ("xv", [256, 2048], bf16),
        ("wqt", [256, 128], bf16),
        ("wkt", [256, 128], bf16),
        ("wvt", [256, 128], bf16),
        ("bq", [128, 1], f32),
        ("bk", [128, 1], f32),
        ("bv", [1, 128], bf16),
        ("wmt0", [64, 256], bf16),
        ("wmt1", [64, 256], bf16),
    ):
        io[name] = nc.dram_tensor(name, shape, dt_, kind="ExternalInput").ap()
    io["out"] = nc.dram_tensor("out", [256, 2048], bf16, kind="ExternalOutput").ap()
    if debug_dumps:
        io["dbg_q"] = nc.dram_tensor("dbg_q", [128, 2048], bf16, kind="ExternalOutput").ap()
        io["dbg_k"] = nc.dram_tensor("dbg_k", [128, 2048], bf16, kind="ExternalOutput").ap()
        io["dbg_vt"] = nc.dram_tensor("dbg_vt", [128, MB, 2, 128], fp8, kind="ExternalOutput").ap()
        io["dbg_x0"] = nc.dram_tensor("dbg_x0", [64, 2048], bf16, kind="ExternalOutput").ap()
        io["dbg_x1"] = nc.dram_tensor("dbg_x1", [64, 2048], bf16, kind="ExternalOutput").ap()

    with tile.TileContext(nc) as tc:
        with ExitStack() as ctx:
            _emit(ctx, tc, io)
    nc.compile()
    _CACHE[key] = nc
    _CACHE[(key, "io")] = io
    return nc


def make_in_maps(query, key, value, wq, bq, wk, bk, wv, bv, wm, bm):
    fb = lambda a: np.ascontiguousarray(np.asarray(a, dtype=np.float32)).astype(BF)
    f = lambda a: np.ascontiguousarray(np.asarray(a), dtype=np.float32)
    query, key, value = f(query), f(key), f(value)
    wq, wk, wv, wm = f(wq), f(wk), f(wv), f(wm)
    bq, bk, bv = f(bq), f(bk), f(bv)
    in_maps = []
    for c in range(8):
        b, pair = divmod(c, 2)
        hs = (2 * pair, 2 * pair + 1)
        idx = np.array([d * H + h for h in hs for d in range(DIM)])
        m = {
            "xq": fb(query[b]),
            "xk": fb(key[b]),
            "xv": fb(value[b]),
            "wqt": fb(wq[idx].T),
            "wkt": fb(wk[idx].T),
            "wvt": fb(wv[idx].T),
            "bq": f(bq[idx].reshape(128, 1)),
            "bk": f(bk[idx].reshape(128, 1)),
            "bv": fb(bv[idx].reshape(1, 128)),
            "wmt0": fb(wm[:, idx[:64]].T),
            "wmt1": fb(wm[:, idx[64:]].T),
        }
        in_maps.append(m)
    return in_maps


def run(in_maps, trace=False, **kw):
    from concourse import bass_utils

    nc = _build_nc()
    return bass_utils.run_bass_kernel_spmd(
        nc, in_maps, core_ids=list(range(8)), trace=trace, **kw
    )


def gather(results, bm):
    bm = np.asarray(bm, dtype=np.float32)
    outs = [np.asarray(r["out"], dtype=np.float32) for r in results]
    return np.stack([outs[2 * b] + outs[2 * b + 1] + bm[:, None] for b in range(B)])


def kernel(query, key, value, wq, bq, wk, bk, wv, bv, wm, bm):
    in_maps = make_in_maps(query, key, value, wq, bq, wk, bk, wv, bv, wm, bm)
    res = run(in_maps)
    return gather(res.results, bm)


# revision 24
# speedup vs baseline: 1.2599x; 1.2599x over previous
"""MultiHeadedAttention Trainium2 Bass kernel (v2: fp8-DR PV + split exp).

Full inputs in, full output out. Sharding: 8 cores = 4 batches x 2 head-pairs.
Per core (1 batch, 2 heads):
  Q/K projections bf16 -> [128, 2048] bf16 SBUF (ACT copy w/ bias, fp32 PSUM)
  V^T projection -> vt [128 m, 16 mb, 2 h, 128] fp8e4m3 where cols 0:64 = v^T
  and cols 64:128 = ones (softmax sums come out of the PV matmul pre-broadcast
  to 64 partitions -- zero extra cycles, matmul cost is N-driven).
  Attention per (n-window 512, head): m-loop over 8 mb-pairs:
    scores bf16 [K=64, M=128, N=512] x2 -> PSUM [128, 2, 512]
    exp: split between ACT (native Exp, fp8 out, scale=1/8) and DVE
    (exp2 bit-trick: i8 = round(s*log2e*8/8 + C2), bits reinterpreted as
    e4m3 = 2^((i8-56)/8) ~ exp(s/8), one tensor_scalar per tile)
    PV-accum: fp8 DoubleRow matmul, K=2x128, accumulated over 8 pairs.
  normalize: single DVE divide px[0:64]/px[64:128] -> x bf16
  out projection bf16, heads accumulated in PSUM; bf16 partial out.
Host sums the two per-batch partials and adds the output bias in fp32.
"""

import sys

if "/opt/trn_rl_repo" not in sys.path:
    sys.path.insert(0, "/opt/trn_rl_repo")

import numpy as np
import ml_dtypes

BF = ml_dtypes.bfloat16

B, D, N, H = 4, 256, 2048, 4
DIM = D // H  # 64
NW = 4  # 512-wide n windows
MB = 16  # 128-wide m blocks

# exp2 bit-trick constant: i8 = round(s * log2(e) + C2); tuned on the
# reference distribution (see transcript prototype).
C1 = 1.4426950408889634
C2 = 55.75

# exp engine pattern per (nw, h) block of 8 mb-pairs: True = ACT, False = DVE.
# j=7 kept on DVE so the div (DVE) queued right after never head-blocks.
PAT_EVEN = [True, False, True, False, True, True, True, False]   # 5A/3D
PAT_ODD = [True, False, True, False, True, False, True, False]   # 4A/4D

_CACHE = {}


def _emit(ctx, tc, io):
    import concourse.bass as bass
    import concourse.mybir as mybir

    nc = tc.nc
    f32 = mybir.dt.float32
    bf16 = mybir.dt.bfloat16
    fp8 = mybir.dt.float8e4
    i8 = mybir.dt.int8
    EXP = mybir.ActivationFunctionType.Exp
    COPY = mybir.ActivationFunctionType.Copy
    IDENT = mybir.ActivationFunctionType.Identity
    DR = mybir.MatmulPerfMode.DoubleRow
    DIVIDE = mybir.AluOpType.divide

    const = ctx.enter_context(tc.tile_pool(name="const", bufs=1))
    xin = ctx.enter_context(tc.tile_pool(name="xin", bufs=4))
    big = ctx.enter_context(tc.tile_pool(name="big", bufs=1))
    ptp = ctx.enter_context(tc.tile_pool(name="probs", bufs=6))
    otp = ctx.enter_context(tc.tile_pool(name="outp", bufs=2))
    psS = ctx.enter_context(tc.tile_pool(name="psS", bufs=3, space="PSUM"))
    psX = ctx.enter_context(tc.tile_pool(name="psX", bufs=2, space="PSUM"))

    # ---- constants / weights ----
    wqt_sb = const.tile([128, 2, 128], bf16, tag="wqt")
    nc.sync.dma_start(wqt_sb, io["wqt"].rearrange("(c p) o -> p c o", p=128))
    wkt_sb = const.tile([128, 2, 128], bf16, tag="wkt")
    nc.sync.dma_start(wkt_sb, io["wkt"].rearrange("(c p) o -> p c o", p=128))
    wvt_sb = const.tile([128, 2, 128], bf16, tag="wvt")
    nc.sync.dma_start(wvt_sb, io["wvt"].rearrange("(c p) o -> p c o", p=128))
    wmt0_sb = const.tile([64, 256], bf16, tag="wmt0")
    nc.sync.dma_start(wmt0_sb, io["wmt0"])
    wmt1_sb = const.tile([64, 256], bf16, tag="wmt1")
    nc.sync.dma_start(wmt1_sb, io["wmt1"])
    bq_sb = const.tile([128, 1], f32, tag="bq")
    nc.sync.dma_start(bq_sb, io["bq"])
    bk_sb = const.tile([128, 1], f32, tag="bk")
    nc.sync.dma_start(bk_sb, io["bk"])
    bv_sb = const.tile([1, 128], bf16, tag="bv")
    nc.sync.dma_start(bv_sb, io["bv"])
    onesb = const.tile([1, 128], bf16, tag="onesb")
    nc.gpsimd.memset(onesb, 1.0)

    # v^T in fp8: [m-in-block, mb, head, col]; cols 64:128 are the ones block
    # that makes the PV matmul emit softmax sums broadcast to partitions 64:128.
    vt = big.tile([128, MB, 2, 128], fp8, tag="vt")
    nc.gpsimd.memset(vt[:, :, :, 64:128], 1.0)

    # PE warmup: release the HAM clock gate during the input-DMA ramp.
    wu_a = const.tile([128, 128], bf16, tag="wu_a")
    nc.gpsimd.memset(wu_a, 0.0)
    wu_b = const.tile([128, 512], bf16, tag="wu_b")
    nc.gpsimd.memset(wu_b, 0.0)
    wu_ps = psS.tile([128, 2, 512], f32, tag="ps", name="wu_ps")
    for _ in range(16):
        nc.tensor.matmul(wu_ps[:, 0, :], lhsT=wu_a, rhs=wu_b, start=True, stop=True)

    # ---- input loads ----
    xq_t, xk_t, xv_t = [], [], []
    eng = {"xq": nc.sync, "xk": nc.gpsimd, "xv": nc.sync}
    for w in range(NW):
        for name, lst in (("xk", xk_t), ("xq", xq_t), ("xv", xv_t)):
            t = xin.tile([128, 2, 512], bf16, tag=name, name=f"{name}{w}")
            src = io[name].rearrange("(c p) n -> p c n", p=128)
            for hh in range(2):
                s = slice(w * 512 + hh * 256, w * 512 + (hh + 1) * 256)
                eng[name].dma_start(t[:, :, hh * 256 : (hh + 1) * 256], src[:, :, s])
            lst.append(t)

    # ---- projections ----
    q_sb = big.tile([128, 2048], bf16, tag="q")
    k_sb = big.tile([128, 2048], bf16, tag="k")

    def proj_pair(xt, wt, bias, dst, wlo):
        # two 512-n windows into one [128, 2, 512] psum tile, one ACT copy
        ps = psS.tile([128, 2, 512], f32, tag="ps", name=f"psproj{wlo}")
        for i, w in enumerate((wlo, wlo + 1)):
            nc.tensor.matmul(ps[:, i, :], lhsT=wt[:, 0, :], rhs=xt[w][:, 0, :],
                             start=True, stop=False)
            nc.tensor.matmul(ps[:, i, :], lhsT=wt[:, 1, :], rhs=xt[w][:, 1, :],
                             start=False, stop=True)
        nc.scalar.activation(
            dst[:, wlo * 512 : (wlo + 2) * 512].rearrange("p (i n) -> p i n", i=2),
            ps, IDENT, bias=bias)

    def vt_group(g):
        # 4 m-blocks (one xv window) -> one [128, 512] psum tile -> 2 fp8 copies
        grp = psX.tile([128, 512], f32, tag="px", name=f"psvt{g}")
        for i in range(4):
            pvt = grp[:, i * 128 : (i + 1) * 128]
            nc.tensor.matmul(pvt, lhsT=onesb, rhs=bv_sb, start=True, stop=False)
            nc.tensor.matmul(pvt, lhsT=xv_t[g][:, 0, i * 128 : (i + 1) * 128],
                             rhs=wvt_sb[:, 0, :], start=False, stop=False)
            nc.tensor.matmul(pvt, lhsT=xv_t[g][:, 1, i * 128 : (i + 1) * 128],
                             rhs=wvt_sb[:, 1, :], start=False, stop=True)
        gview = grp.rearrange("p (i c) -> p i c", i=4)
        for h in range(2):
            nc.scalar.activation(vt[:, 4 * g : 4 * g + 4, h, 0:64],
                                 gview[:, :, h * 64 : (h + 1) * 64], COPY)

    # ---- attention ----
    xh = [big.tile([64, 2048], bf16, tag=f"x{h}", name=f"x{h}") for h in range(2)]
    pend_div = [None]

    def flush_div():
        if pend_div[0] is not None:
            h, nw, px = pend_div[0]
            # sums sit pre-broadcast in px[64:128]; DVE has one PSUM read port
            # and no divide, so: ACT copies sums to SBUF, idle GPSIMD takes
            # the reciprocal (pow -1), DVE multiplies.
            s_sb = otp.tile([64, 512], f32, tag="s", name=f"s{h}_{nw}")
            nc.scalar.activation(s_sb, px[64:128, :], COPY)
            r_sb = otp.tile([64, 512], f32, tag="r", name=f"r{h}_{nw}")
            nc.vector.reciprocal_approx_fast(r_sb, s_sb)
            nc.vector.tensor_tensor(
                xh[h][:, nw * 512 : (nw + 1) * 512],
                px[0:64, :], r_sb, op=mybir.AluOpType.mult)
            pend_div[0] = None

    def make_block(nw, h, blk):
        return {"nw": nw, "h": h,
                "pat": PAT_EVEN if blk % 2 == 0 else PAT_ODD,
                "px": psX.tile([128, 512], f32, tag="px", name=f"px{blk}"),
                "pts": [], "blk": blk}

    def burst(n):
        # dense junk matmuls (no data deps beyond pool rotation): a contiguous
        # busy stretch that releases / holds the HAM 2.4GHz clock gate.
        bt = psS.tile([128, 2, 512], f32, tag="ps", name="burst")
        for _ in range(n):
            nc.tensor.matmul(bt[:, 0, :], lhsT=wu_a, rhs=wu_b,
                             start=True, stop=True)

    def attn_pairs(st, j_lo, j_hi):
        nw, h, blk, px = st["nw"], st["h"], st["blk"], st["px"]
        kh = k_sb[h * 64 : (h + 1) * 64, :]
        qh = q_sb[h * 64 : (h + 1) * 64, nw * 512 : (nw + 1) * 512]
        for j in range(j_lo, j_hi):
            sc = psS.tile([128, 2, 512], f32, tag="ps", name=f"sc{blk}_{j}")
            st.setdefault("scs", []).append(sc)
            for kt in range(2):
                mb = 2 * j + kt
                nc.tensor.matmul(sc[:, kt, :],
                                 lhsT=kh[:, mb * 128 : (mb + 1) * 128], rhs=qh,
                                 start=True, stop=True)
            pt = ptp.tile([128, 2, 512], fp8, tag="pt", name=f"pt{blk}_{j}")
            if st["pat"][j]:
                nc.scalar.activation(pt, sc, EXP, scale=0.125)
            else:
                nc.vector.tensor_scalar(pt.bitcast(i8), sc, C1, C2,
                                        op0=mybir.AluOpType.mult,
                                        op1=mybir.AluOpType.add)
            st["pts"].append(pt)
            if j == 1:
                flush_div()
            if j >= 2:
                jj = j - 2
                nc.tensor.matmul(px, lhsT=vt[:, 2 * jj : 2 * jj + 2, h, :],
                                 rhs=st["pts"][jj], start=(j == 2), stop=False,
                                 perf_mode=DR, skip_group_check=True)

    def attn_tail(st):
        h, px = st["h"], st["px"]
        for j in (6, 7):
            nc.tensor.matmul(px, lhsT=vt[:, 2 * j : 2 * j + 2, h, :],
                             rhs=st["pts"][j], start=False, stop=(j == 7),
                             perf_mode=DR, skip_group_check=True)
        burst(2)
        pend_div[0] = (st["h"], st["nw"], px)

    def attn_block(nw, h, blk):
        st = make_block(nw, h, blk)
        attn_pairs(st, 0, 8)
        attn_tail(st)

    def out_proj(half):
        for oc in range(2):
            po = psS.tile([128, 2, 512], f32, tag="ps", name=f"po{half}_{oc}")
            ocs = slice(oc * 128, (oc + 1) * 128)
            for s in range(2):
                ns = slice(half * 1024 + s * 512, half * 1024 + (s + 1) * 512)
                nc.tensor.matmul(po[:, s, :], lhsT=wmt0_sb[:, ocs],
                                 rhs=xh[0][:, ns], start=True, stop=False)
                nc.tensor.matmul(po[:, s, :], lhsT=wmt1_sb[:, ocs],
                                 rhs=xh[1][:, ns], start=False, stop=True)
            ot = otp.tile([128, 1024], bf16, tag="ot", name=f"ot{half}_{oc}")
            if oc == 0:
                nc.scalar.activation(ot, po.rearrange("p i n -> p (i n)"), COPY)
            else:
                nc.vector.tensor_copy(ot, po.rearrange("p i n -> p (i n)"))
            nc.sync.dma_start(
                io["out"][ocs, half * 1024 : (half + 1) * 1024], ot)

    proj_pair(xk_t, wkt_sb, bk_sb, k_sb, 0)
    proj_pair(xq_t, wqt_sb, bq_sb, q_sb, 0)
    vt_group(0)
    proj_pair(xk_t, wkt_sb, bk_sb, k_sb, 2)
    proj_pair(xq_t, wqt_sb, bq_sb, q_sb, 2)
    vt_group(1)
    vt_group(2)
    vt_group(3)
    burst(10)

    blk = 0
    for nw in range(NW):
        for h in range(2):
            attn_block(nw, h, blk)
            blk += 1
        if nw == 1:
            flush_div()
            out_proj(0)
    flush_div()
    out_proj(1)

    if "dbg_q" in io:
        nc.sync.dma_start(io["dbg_q"], q_sb)
        nc.sync.dma_start(io["dbg_k"], k_sb)
        nc.sync.dma_start(io["dbg_vt"], vt)
        nc.sync.dma_start(io["dbg_x0"], xh[0])
        nc.sync.dma_start(io["dbg_x1"], xh[1])


def _build_nc(debug_dumps=False):
    key = ("nc", debug_dumps)
    if key in _CACHE:
        return _CACHE[key]
    from contextlib import ExitStack

    import concourse.mybir as mybir
    import concourse.tile as tile
    from concourse import bacc

    f32 = mybir.dt.float32
    bf16 = mybir.dt.bfloat16
    fp8 = mybir.dt.float8e4
    nc = bacc.Bacc("TRN2", target_bir_lowering=False, debug=False, num_devices=8)
    io = {}
    for name, shape, dt_ in (
        ("xq", [256, 2048], bf16),
        ("xk", [256, 2048], bf16),
        ("xv", [256, 2048], bf16),
        ("wqt", [256, 128], bf16),
        ("wkt", [256, 128], bf16),
        ("wvt", [256, 128], bf16),
        ("bq", [128, 1], f32),
        ("bk", [128, 1], f32),
        ("bv", [1, 128], bf16),
        ("wmt0", [64, 256], bf16),
        ("wmt1", [64, 256], bf16),
    ):
        io[name] = nc.dram_tensor(name, shape, dt_, kind="ExternalInput").ap()
    io["out"] = nc.dram_tensor("out", [256, 2048], bf16, kind="ExternalOutput").ap()
    if debug_dumps:
        io["dbg_q"] = nc.dram_tensor("dbg_q", [128, 2048], bf16, kind="ExternalOutput").ap()
        io["dbg_k"] = nc.dram_tensor("dbg_k", [128, 2048], bf16, kind="ExternalOutput").ap()
        io["dbg_vt"] = nc.dram_tensor("dbg_vt", [128, MB, 2, 128], fp8, kind="ExternalOutput").ap()
        io["dbg_x0"] = nc.dram_tensor("dbg_x0", [64, 2048], bf16, kind="ExternalOutput").ap()
        io["dbg_x1"] = nc.dram_tensor("dbg_x1", [64, 2048], bf16, kind="ExternalOutput").ap()

    with tile.TileContext(nc) as tc:
        with ExitStack() as ctx:
            _emit(ctx, tc, io)
    nc.compile()
    _CACHE[key] = nc
    _CACHE[(key, "io")] = io
    return nc


def make_in_maps(query, key, value, wq, bq, wk, bk, wv, bv, wm, bm):
    fb = lambda a: np.ascontiguousarray(np.asarray(a, dtype=np.float32)).astype(BF)
    f = lambda a: np.ascontiguousarray(np.asarray(a), dtype=np.float32)
    query, key, value = f(query), f(key), f(value)
    wq, wk, wv, wm = f(wq), f(wk), f(wv), f(wm)
    bq, bk, bv = f(bq), f(bk), f(bv)
    in_maps = []
    for c in range(8):
        b, pair = divmod(c, 2)
        hs = (2 * pair, 2 * pair + 1)
        idx = np.array([d * H + h for h in hs for d in range(DIM)])
        m = {
            "xq": fb(query[b]),
            "xk": fb(key[b]),
            "xv": fb(value[b]),
            "wqt": fb(wq[idx].T),
            "wkt": fb(wk[idx].T),
            "wvt": fb(wv[idx].T),
            "bq": f(bq[idx].reshape(128, 1)),
            "bk": f(bk[idx].reshape(128, 1)),
            "bv": fb(bv[idx].reshape(1, 128)),
            "wmt0": fb(wm[:, idx[:64]].T),
            "wmt1": fb(wm[:, idx[64:]].T),
        }
        in_maps.append(m)
    return in_maps


def run(in_maps, trace=False, **kw):
    from concourse import bass_utils

    nc = _build_nc()
    return bass_utils.run_bass_kernel_spmd(
        nc, in_maps, core_ids=list(range(8)), trace=trace, **kw
    )


def gather(results, bm):
    bm = np.asarray(bm, dtype=np.float32)
    outs = [np.asarray(r["out"], dtype=np.float32) for r in results]
    return np.stack([outs[2 * b] + outs[2 * b + 1] + bm[:, None] for b in range(B)])


def kernel(query, key, value, wq, bq, wk, bk, wv, bv, wm, bm):
    in_maps = make_in_maps(query, key, value, wq, bq, wk, bk, wv, bv, wm, bm)
    res = run(in_maps)
    return gather(res.results, bm)


# revision 25
# speedup vs baseline: 1.5450x; 1.2263x over previous
"""MultiHeadedAttention Trainium2 Bass kernel (v2: fp8-DR PV + split exp).

Full inputs in, full output out. Sharding: 8 cores = 4 batches x 2 head-pairs.
Per core (1 batch, 2 heads):
  Q/K projections bf16 -> [128, 2048] bf16 SBUF (ACT copy w/ bias, fp32 PSUM)
  V^T projection -> vt [128 m, 16 mb, 2 h, 128] fp8e4m3 where cols 0:64 = v^T
  and cols 64:128 = ones (softmax sums come out of the PV matmul pre-broadcast
  to 64 partitions -- zero extra cycles, matmul cost is N-driven).
  Attention per (n-window 512, head): m-loop over 8 mb-pairs:
    scores bf16 [K=64, M=128, N=512] x2 -> PSUM [128, 2, 512]
    exp: split between ACT (native Exp, fp8 out, scale=1/8) and DVE
    (exp2 bit-trick: i8 = round(s*log2e*8/8 + C2), bits reinterpreted as
    e4m3 = 2^((i8-56)/8) ~ exp(s/8), one tensor_scalar per tile)
    PV-accum: fp8 DoubleRow matmul, K=2x128, accumulated over 8 pairs.
  normalize: single DVE divide px[0:64]/px[64:128] -> x bf16
  out projection bf16, heads accumulated in PSUM; bf16 partial out.
Host sums the two per-batch partials and adds the output bias in fp32.
"""

import sys

if "/opt/trn_rl_repo" not in sys.path:
    sys.path.insert(0, "/opt/trn_rl_repo")

import numpy as np
import ml_dtypes

BF = ml_dtypes.bfloat16

B, D, N, H = 4, 256, 2048, 4
DIM = D // H  # 64
NW = 4  # 512-wide n windows
MB = 16  # 128-wide m blocks

# exp2 bit-trick constant: i8 = round(s * log2(e) + C2); tuned on the
# reference distribution (see transcript prototype).
C1 = 1.4426950408889634
C2 = 55.75

# exp engine pattern per (nw, h) block of 8 mb-pairs: True = ACT, False = DVE.
# j=7 kept on DVE so the div (DVE) queued right after never head-blocks.
PAT_EVEN = [True, False, True, False, True, True, True, False]   # 5A/3D
PAT_ODD = [True, False, True, False, True, False, True, False]   # 4A/4D

_CACHE = {}


def _emit(ctx, tc, io):
    import concourse.bass as bass
    import concourse.mybir as mybir

    nc = tc.nc
    f32 = mybir.dt.float32
    bf16 = mybir.dt.bfloat16
    fp8 = mybir.dt.float8e4
    i8 = mybir.dt.int8
    EXP = mybir.ActivationFunctionType.Exp
    COPY = mybir.ActivationFunctionType.Copy
    IDENT = mybir.ActivationFunctionType.Identity
    DR = mybir.MatmulPerfMode.DoubleRow
    DIVIDE = mybir.AluOpType.divide

    const = ctx.enter_context(tc.tile_pool(name="const", bufs=1))
    xin = ctx.enter_context(tc.tile_pool(name="xin", bufs=4))
    big = ctx.enter_context(tc.tile_pool(name="big", bufs=1))
    ptp = ctx.enter_context(tc.tile_pool(name="probs", bufs=6))
    otp = ctx.enter_context(tc.tile_pool(name="outp", bufs=2))
    psS = ctx.enter_context(tc.tile_pool(name="psS", bufs=3, space="PSUM"))
    psX = ctx.enter_context(tc.tile_pool(name="psX", bufs=2, space="PSUM"))

    # ---- constants / weights ----
    wqt_sb = const.tile([128, 2, 128], bf16, tag="wqt")
    nc.sync.dma_start(wqt_sb, io["wqt"].rearrange("(c p) o -> p c o", p=128))
    wkt_sb = const.tile([128, 2, 128], bf16, tag="wkt")
    nc.sync.dma_start(wkt_sb, io["wkt"].rearrange("(c p) o -> p c o", p=128))
    wvt_sb = const.tile([128, 2, 128], bf16, tag="wvt")
    nc.sync.dma_start(wvt_sb, io["wvt"].rearrange("(c p) o -> p c o", p=128))
    wmt0_sb = const.tile([64, 256], bf16, tag="wmt0")
    nc.sync.dma_start(wmt0_sb, io["wmt0"])
    wmt1_sb = const.tile([64, 256], bf16, tag="wmt1")
    nc.sync.dma_start(wmt1_sb, io["wmt1"])
    bq_sb = const.tile([128, 1], f32, tag="bq")
    nc.sync.dma_start(bq_sb, io["bq"])
    bk_sb = const.tile([128, 1], f32, tag="bk")
    nc.sync.dma_start(bk_sb, io["bk"])
    bv_sb = const.tile([1, 128], bf16, tag="bv")
    nc.sync.dma_start(bv_sb, io["bv"])
    onesb = const.tile([1, 128], bf16, tag="onesb")
    nc.gpsimd.memset(onesb, 1.0)

    # v^T in fp8: [m-in-block, mb, head, col]; cols 64:128 are the ones block
    # that makes the PV matmul emit softmax sums broadcast to partitions 64:128.
    vt = big.tile([128, MB, 2, 128], fp8, tag="vt")
    nc.gpsimd.memset(vt[:, :, :, 64:128], 1.0)

    # PE warmup: release the HAM clock gate during the input-DMA ramp.
    wu_a = const.tile([128, 128], bf16, tag="wu_a")
    nc.gpsimd.memset(wu_a, 0.0)
    wu_b = const.tile([128, 512], bf16, tag="wu_b")
    nc.gpsimd.memset(wu_b, 0.0)
    wu_ps = psS.tile([128, 2, 512], f32, tag="ps", name="wu_ps")
    for _ in range(16):
        nc.tensor.matmul(wu_ps[:, 0, :], lhsT=wu_a, rhs=wu_b, start=True, stop=True)

    # ---- input loads ----
    xq_t, xk_t, xv_t = [], [], []
    eng = {"xq": nc.sync, "xk": nc.gpsimd, "xv": nc.sync}
    for w in range(NW):
        for name, lst in (("xk", xk_t), ("xq", xq_t), ("xv", xv_t)):
            t = xin.tile([128, 2, 512], bf16, tag=name, name=f"{name}{w}")
            src = io[name].rearrange("(c p) n -> p c n", p=128)
            for hh in range(2):
                s = slice(w * 512 + hh * 256, w * 512 + (hh + 1) * 256)
                eng[name].dma_start(t[:, :, hh * 256 : (hh + 1) * 256], src[:, :, s])
            lst.append(t)

    # ---- projections ----
    q_sb = big.tile([128, 2048], bf16, tag="q")
    k_sb = big.tile([128, 2048], bf16, tag="k")

    def proj_pair(xt, wt, bias, dst, wlo):
        # two 512-n windows into one [128, 2, 512] psum tile, one ACT copy
        ps = psS.tile([128, 2, 512], f32, tag="ps", name=f"psproj{wlo}")
        for i, w in enumerate((wlo, wlo + 1)):
            nc.tensor.matmul(ps[:, i, :], lhsT=wt[:, 0, :], rhs=xt[w][:, 0, :],
                             start=True, stop=False)
            nc.tensor.matmul(ps[:, i, :], lhsT=wt[:, 1, :], rhs=xt[w][:, 1, :],
                             start=False, stop=True)
        nc.scalar.activation(
            dst[:, wlo * 512 : (wlo + 2) * 512].rearrange("p (i n) -> p i n", i=2),
            ps, IDENT, bias=bias)

    def vt_group(g):
        # 4 m-blocks (one xv window) -> one [128, 512] psum tile -> 2 fp8 copies
        grp = psX.tile([128, 512], f32, tag="px", name=f"psvt{g}")
        for i in range(4):
            pvt = grp[:, i * 128 : (i + 1) * 128]
            nc.tensor.matmul(pvt, lhsT=onesb, rhs=bv_sb, start=True, stop=False)
            nc.tensor.matmul(pvt, lhsT=xv_t[g][:, 0, i * 128 : (i + 1) * 128],
                             rhs=wvt_sb[:, 0, :], start=False, stop=False)
            nc.tensor.matmul(pvt, lhsT=xv_t[g][:, 1, i * 128 : (i + 1) * 128],
                             rhs=wvt_sb[:, 1, :], start=False, stop=True)
        gview = grp.rearrange("p (i c) -> p i c", i=4)
        for h in range(2):
            nc.scalar.activation(vt[:, 4 * g : 4 * g + 4, h, 0:64],
                                 gview[:, :, h * 64 : (h + 1) * 64], COPY)

    # ---- attention ----
    xh = [big.tile([64, 2048], bf16, tag=f"x{h}", name=f"x{h}") for h in range(2)]
    pend_div = [None]

    def flush_div():
        if pend_div[0] is not None:
            h, nw, px = pend_div[0]
            # sums sit pre-broadcast in px[64:128]; DVE has one PSUM read port
            # and no divide, so: ACT copies sums to SBUF, idle GPSIMD takes
            # the reciprocal (pow -1), DVE multiplies.
            s_sb = otp.tile([64, 512], f32, tag="s", name=f"s{h}_{nw}")
            nc.scalar.activation(s_sb, px[64:128, :], COPY)
            r_sb = otp.tile([64, 512], f32, tag="r", name=f"r{h}_{nw}")
            nc.vector.reciprocal_approx_fast(r_sb, s_sb)
            nc.vector.tensor_tensor(
                xh[h][:, nw * 512 : (nw + 1) * 512],
                px[0:64, :], r_sb, op=mybir.AluOpType.mult)
            pend_div[0] = None

    def make_block(nw, h, blk):
        return {"nw": nw, "h": h,
                "pat": PAT_EVEN if blk % 2 == 0 else PAT_ODD,
                "px": psX.tile([128, 512], f32, tag="px", name=f"px{blk}"),
                "pts": [], "blk": blk}

    def burst(n):
        # dense junk matmuls (no data deps beyond pool rotation): a contiguous
        # busy stretch that releases / holds the HAM 2.4GHz clock gate.
        bt = psS.tile([128, 2, 512], f32, tag="ps", name="burst")
        for _ in range(n):
            nc.tensor.matmul(bt[:, 0, :], lhsT=wu_a, rhs=wu_b,
                             start=True, stop=True)

    def attn_pairs(st, j_lo, j_hi):
        nw, h, blk, px = st["nw"], st["h"], st["blk"], st["px"]
        kh = k_sb[h * 64 : (h + 1) * 64, :]
        qh = q_sb[h * 64 : (h + 1) * 64, nw * 512 : (nw + 1) * 512]
        for j in range(j_lo, j_hi):
            sc = psS.tile([128, 2, 512], f32, tag="ps", name=f"sc{blk}_{j}")
            st.setdefault("scs", []).append(sc)
            for kt in range(2):
                mb = 2 * j + kt
                nc.tensor.matmul(sc[:, kt, :],
                                 lhsT=kh[:, mb * 128 : (mb + 1) * 128], rhs=qh,
                                 start=True, stop=True)
            pt = ptp.tile([128, 2, 512], fp8, tag="pt", name=f"pt{blk}_{j}")
            if st["pat"][j]:
                nc.scalar.activation(pt, sc, EXP, scale=0.125)
            else:
                nc.vector.tensor_scalar(pt.bitcast(i8), sc, C1, C2,
                                        op0=mybir.AluOpType.mult,
                                        op1=mybir.AluOpType.add)
            st["pts"].append(pt)
            if j == 1:
                flush_div()
            if j == 3 and st["blk"] < 2:
                burst(2)
            if j >= 2:
                jj = j - 2
                nc.tensor.matmul(px, lhsT=vt[:, 2 * jj : 2 * jj + 2, h, :],
                                 rhs=st["pts"][jj], start=(j == 2), stop=False,
                                 perf_mode=DR, skip_group_check=True)

    def attn_tail(st):
        h, px = st["h"], st["px"]
        for j in (6, 7):
            nc.tensor.matmul(px, lhsT=vt[:, 2 * j : 2 * j + 2, h, :],
                             rhs=st["pts"][j], start=False, stop=(j == 7),
                             perf_mode=DR, skip_group_check=True)
        burst(2)
        pend_div[0] = (st["h"], st["nw"], px)

    def attn_block(nw, h, blk):
        st = make_block(nw, h, blk)
        attn_pairs(st, 0, 8)
        attn_tail(st)

    def out_proj(half):
        for oc in range(2):
            po = psS.tile([128, 2, 512], f32, tag="ps", name=f"po{half}_{oc}")
            ocs = slice(oc * 128, (oc + 1) * 128)
            for s in range(2):
                ns = slice(half * 1024 + s * 512, half * 1024 + (s + 1) * 512)
                nc.tensor.matmul(po[:, s, :], lhsT=wmt0_sb[:, ocs],
                                 rhs=xh[0][:, ns], start=True, stop=False)
                nc.tensor.matmul(po[:, s, :], lhsT=wmt1_sb[:, ocs],
                                 rhs=xh[1][:, ns], start=False, stop=True)
            ot = otp.tile([128, 1024], bf16, tag="ot", name=f"ot{half}_{oc}")
            if oc == 0:
                nc.scalar.activation(ot, po.rearrange("p i n -> p (i n)"), COPY)
            else:
                nc.vector.tensor_copy(ot, po.rearrange("p i n -> p (i n)"))
            nc.sync.dma_start(
                io["out"][ocs, half * 1024 : (half + 1) * 1024], ot)

    proj_pair(xk_t, wkt_sb, bk_sb, k_sb, 0)
    proj_pair(xq_t, wqt_sb, bq_sb, q_sb, 0)
    vt_group(0)
    proj_pair(xk_t, wkt_sb, bk_sb, k_sb, 2)
    proj_pair(xq_t, wqt_sb, bq_sb, q_sb, 2)
    vt_group(1)
    vt_group(2)
    vt_group(3)
    burst(10)

    blk = 0
    for nw in range(NW):
        for h in range(2):
            attn_block(nw, h, blk)
            blk += 1
        if nw == 1:
            flush_div()
            burst(8)
            out_proj(0)
    flush_div()
    burst(8)
    out_proj(1)

    if "dbg_q" in io:
        nc.sync.dma_start(io["dbg_q"], q_sb)
        nc.sync.dma_start(io["dbg_k"], k_sb)
        nc.sync.dma_start(io["dbg_vt"], vt)
        nc.sync.dma_start(io["dbg_x0"], xh[0])
        nc.sync.dma_start(io["dbg_x1"], xh[1])


def _build_nc(debug_dumps=False):
    key = ("nc", debug_dumps)
    if key in _CACHE:
        return _CACHE[key]
    from contextlib import ExitStack

    import concourse.mybir as mybir
    import concourse.tile as tile
    from concourse import bacc

    f32 = mybir.dt.float32
    bf16 = mybir.dt.bfloat16
    fp8 = mybir.dt.float8e4
    nc = bacc.Bacc("TRN2", target_bir_lowering=False, debug=False, num_devices=8)
    io = {}
    for name, shape, dt_ in (
        ("xq", [256, 2048], bf16),
        ("xk", [256, 2048], bf16),
        ("xv", [256, 2048], bf16),
        ("wqt", [256, 128], bf16),
        ("wkt", [256, 128], bf16),
        ("wvt", [256, 128], bf16),
        ("bq", [128, 1], f32),
        ("bk", [128, 1], f32),
        ("bv", [1, 128], bf16),
        ("wmt0", [64, 256], bf16),
        ("wmt1", [64, 256], bf16),
    ):
        io[name] = nc.dram_tensor(name, shape, dt_, kind="ExternalInput").ap()
    io["out"] = nc.dram_tensor("out", [256, 2048], bf16, kind="ExternalOutput").ap()
    if debug_dumps:
        io["dbg_q"] = nc.dram_tensor("dbg_q", [128, 2048], bf16, kind="ExternalOutput").ap()
        io["dbg_k"] = nc.dram_tensor("dbg_k", [128, 2048], bf16, kind="ExternalOutput").ap()
        io["dbg_vt"] = nc.dram_tensor("dbg_vt", [128, MB, 2, 128], fp8, kind="ExternalOutput").ap()
        io["dbg_x0"] = nc.dram_tensor("dbg_x0", [64, 2048], bf16, kind="ExternalOutput").ap()
        io["dbg_x1"] = nc.dram_tensor("dbg_x1", [64, 2048], bf16, kind="ExternalOutput").ap()

    with tile.TileContext(nc) as tc:
        with ExitStack() as ctx:
            _emit(ctx, tc, io)
    nc.compile()
    _CACHE[key] = nc
    _CACHE[(key, "io")] = io
    return nc


def make_in_maps(query, key, value, wq, bq, wk, bk, wv, bv, wm, bm):
    fb = lambda a: np.ascontiguousarray(np.asarray(a, dtype=np.float32)).astype(BF)
    f = lambda a: np.ascontiguousarray(np.asarray(a), dtype=np.float32)
    query, key, value = f(query), f(key), f(value)
    wq, wk, wv, wm = f(wq), f(wk), f(wv), f(wm)
    bq, bk, bv = f(bq), f(bk), f(bv)
    in_maps = []
    for c in range(8):
        b, pair = divmod(c, 2)
        hs = (2 * pair, 2 * pair + 1)
        idx = np.array([d * H + h for h in hs for d in range(DIM)])
        m = {
            "xq": fb(query[b]),
            "xk": fb(key[b]),
            "xv": fb(value[b]),
            "wqt": fb(wq[idx].T),
            "wkt": fb(wk[idx].T),
            "wvt": fb(wv[idx].T),
            "bq": f(bq[idx].reshape(128, 1)),
            "bk": f(bk[idx].reshape(128, 1)),
            "bv": fb(bv[idx].reshape(1, 128)),
            "wmt0": fb(wm[:, idx[:64]].T),
            "wmt1": fb(wm[:, idx[64:]].T),
        }
        in_maps.append(m)
    return in_maps


def run(in_maps, trace=False, **kw):
    from concourse import bass_utils

    nc = _build_nc()
    return bass_utils.run_bass_kernel_spmd(
        nc, in_maps, core_ids=list(range(8)), trace=trace, **kw
    )


def gather(results, bm):
    bm = np.asarray(bm, dtype=np.float32)
    outs = [np.asarray(r["out"], dtype=np.float32) for r in results]
    return np.stack([outs[2 * b] + outs[2 * b + 1] + bm[:, None] for b in range(B)])


def kernel(query, key, value, wq, bq, wk, bk, wv, bv, wm, bm):
    in_maps = make_in_maps(query, key, value, wq, bq, wk, bk, wv, bv, wm, bm)
    res = run(in_maps)
    return gather(res.results, bm)
